# revision 1
# baseline (speedup 1.0000x reference)
"""Trainium2 Bass kernel for nn_BernConvLayer — fp8 DoubleRow edition.

Math: for the benchmark input coe = ones(11) the Bernstein filter collapses
to identity (see _monomial_coeffs: sum_k C(K,k)/2^K (I-Ahat)^k (I+Ahat)^{K-k}
= I exactly), so the module degenerates to a 4-matmul MLP per graph —
`adj` contributes nothing. One graph per NeuronCore (B=8 = 8 cores), all
compute in transposed space (channels on partitions, nodes on the free dim).

Numerical scheme (max-rel-err 1.36e-2 on HW vs the 2e-2 gate, chosen via a
host-side quantization study — see git history sim_quant*.py):
  - H stages (W0/W1/W2) run as fp8-e4m3 DoubleRow matmuls (0.5 cyc/row,
    two 128-row K-chunks per instruction): activations carry a global
    scale S=16, weights a per-tensor scale 210/max|W|.  ACT applies the
    relu writing h as bf16; DVE casts h0/h1 to fp8 for the next
    stages (h2 needs no fp8 view — nothing downstream consumes it).
  - bern = x_bf16 + h_bf16 (DVE) keeps bern at bf16 precision, then is
    split bern = bern_hi + bern_lo (both fp8 at scale S; hi-cast on DVE,
    lo-subtract split across DVE and GPSIMD).
  - OUT (Wout) also runs fp8-DoubleRow with 3-term compensation
    psum = bern_hi@Whi + bern_hi@Wlo + bern_lo@Whi (Wout hi/lo split on
    the host) — naive fp8 would land ~4.7e-2, over the gate.
  - PSUM stays fp32; casts to bf16 with scale 1/(S*sWo) ride ACT (DVE for
    the last tile); host only de-transposes and casts bf16 -> f32.

Schedule (cost-model timeline 49.6us/core): PE busy 33.6us (78 DoubleRow
matmuls per 512-node tile, 4 tiles). All H stages are emitted first with
tiles' stages interleaved (the ACT queue stays a pure h-chain; OUT casts
queued earlier would head-of-line-block it), then the 12 OUT groups run
back-to-back — PE is gap-free from ~24us to the end.  Loads are issued in
consumption order (HWDGE serializes ~630ns per issue; first w0/x0
pair-chunks ride separate queues as standalone tiles), stores ride the SP
queue (SWDGE on Pool costs ~1us engine time each), and the kernel-tail
drain chain is spread across all five engine queues.
"""

import sys
from math import comb

import numpy as np
import ml_dtypes

for _p in ("/opt/trn_rl_repo", "/root/.axon_site/_ro/trn_rl_repo"):
    if _p not in sys.path:
        sys.path.append(_p)

F8 = ml_dtypes.float8_e4m3
BF16 = ml_dtypes.bfloat16

K = 10
B, N, H = 8, 2048, 768
HD = H // 3
P = 128
NTILE = 512
NT = N // NTILE
S = 16.0       # global activation scale for fp8 storage
WS = 210.0     # weight scale target (fp8 e4m3 max finite is 240)


def _monomial_coeffs(coe: np.ndarray) -> np.ndarray:
    """Exact monomial coefficients a_j of p(s) = sum_k c_k (1-s)^k (1+s)^{K-k}."""
    T = np.maximum(np.asarray(coe, np.float64), 0.0)
    a = np.zeros(K + 1)
    for k in range(K + 1):
        c_k = float(T[k]) * comb(K, k) / 2.0**K
        pa = [comb(k, i) * (-1) ** i for i in range(k + 1)]
        pb = [comb(K - k, i) for i in range(K - k + 1)]
        prod = [0] * (K + 1)
        for i, va in enumerate(pa):
            for j, vb in enumerate(pb):
                prod[i + j] += va * vb
        for j in range(K + 1):
            a[j] += c_k * prod[j]
    return a


_CACHE = {}


def _patch_drain_waits():
    """Walrus rejects instructions with more sync waits than the ISA encoding
    holds; split excess waits onto same-engine NOPs / chained drains."""
    import concourse.mybir as mybir
    from concourse.tile import TileContext
    from concourse.vector_clock import ScopedClock
    import bass_rust

    if getattr(TileContext, "_drain_waits_patched", False):
        return

    _MAXW = 1

    _orig_commit = TileContext._commit_instruction

    def _split_commit_instruction(self, inst, lazy_reg_writes: bool = True):
        si = getattr(inst, "sync_info", None)
        eng = getattr(inst, "engine", None)
        if (
            si is not None
            and len(si.on_wait) > _MAXW
            and eng is not None
            and eng != mybir.EngineType.Unassigned
        ):
            waits = list(si.on_wait)
            while len(waits) > _MAXW:
                chunk, waits = waits[:_MAXW], waits[_MAXW:]
                nop = mybir.InstNoOp(
                    name=self.nc.get_next_instruction_name(),
                    sync_info=mybir.SyncInfo(on_wait=chunk, on_update=[]),
                    bass_nofuse=True,
                    engine=eng,
                )
                _orig_commit(self, nop, lazy_reg_writes=False)
            inst.sync_info = bass_rust.SyncInfo(
                on_wait=waits, on_update=list(si.on_update)
            )
        return _orig_commit(self, inst, lazy_reg_writes)

    TileContext._commit_instruction = _split_commit_instruction

    def _split_drain_and_barrier(self, tick_clock, wait_clock):
        drain_inst = self.nc.sync.drain()
        wait_clock.add_sem_waits(
            drain_inst.ins, ScopedClock({None: tick_clock.global_clock})
        )
        si = drain_inst.ins.sync_info
        if si is not None and len(si.on_wait) > 1:
            waits = list(si.on_wait)
            updates = list(si.on_update)
            drain_inst.ins.sync_info = bass_rust.SyncInfo(
                on_wait=waits[:1], on_update=[]
            )
            rest = waits[1:]
            # spread the single-wait drain chain across all engine queues so
            # the tail pays ~n/5 sequential drains instead of n (the
            # all_engine_barrier below joins them)
            engines = [self.nc.sync, self.nc.scalar, self.nc.vector,
                       self.nc.gpsimd, self.nc.tensor]
            i = 0
            while rest:
                chunk, rest = rest[:1], rest[1:]
                extra = engines[i % len(engines)].drain()
                i += 1
                extra.ins.sync_info = bass_rust.SyncInfo(
                    on_wait=chunk, on_update=updates if not rest else []
                )

        self.nc.all_engine_barrier()
        assert self.sems is not None
        popped = self.nc._tile_sem_poison_stack.pop()
        assert popped is self._sem_poison
        self.nc.clear_and_free_semaphores(list(self.sems.allocated().values()))
        self.nc.all_engine_barrier()

    TileContext._drain_and_barrier = _split_drain_and_barrier
    TileContext._drain_waits_patched = True


def _build_fp8_bass(a0: float, inv_sw, zero_bias: bool, inv_swo: float = 1.0):
    """Per-core MLP: fp8 DoubleRow H stages + compensated fp8 DoubleRow OUT.

    DRAM params (host-prearranged, partition-major):
      xq  [128, 6, 2048] fp8   S * x.T, k-chunk-major
      xb  [128, 6, 2048] bf16  S * x.T
      w0q [128, 6, 256]  fp8   sW0 * W0
      w1q [128, 8, 256]  fp8
      w2q [128, 10, 256] fp8
      whi/wlo [128, 6, 768] fp8  sWo*Wout hi/lo split (lo = residual)
      (non-zero-bias variant adds b0s..b2s [128,2,1], bouts [128,6,1] f32)
      yT  [128, 6, 2048] bf16  output channels-major (out + bout)

    OUT per bank: psum = bern_hi@Whi + bern_hi@Wlo + bern_lo@Whi, where
    bern_bf16 = x_bf16 + h_bf16 (DVE), bern_hi = fp8(bern_bf16) (DVE),
    bern_lo = fp8(bern_bf16 - bern_hi) (GPSIMD). Cast to bf16 with
    scale 1/(S*sWo) splits between ACT and DVE.
    """
    import concourse.bass as bass
    import concourse.mybir as mybir
    from concourse.bass import ts
    from concourse.tile import TileContext

    _patch_drain_waits()

    f32 = mybir.dt.float32
    f8 = mybir.dt.float8e4
    bf = mybir.dt.bfloat16
    AF = mybir.ActivationFunctionType
    DR = mybir.MatmulPerfMode.DoubleRow

    nc = bass.Bass("TRN2", target_bir_lowering=False, debug=False)
    xq = nc.declare_dram_parameter("xq", [P, 6, N], f8, isOutput=False)
    xb = nc.declare_dram_parameter("xb", [P, 6, N], bf, isOutput=False)
    w0q = nc.declare_dram_parameter("w0q", [P, 6, HD], f8, isOutput=False)
    w1q = nc.declare_dram_parameter("w1q", [P, 8, HD], f8, isOutput=False)
    w2q = nc.declare_dram_parameter("w2q", [P, 10, HD], f8, isOutput=False)
    whi = nc.declare_dram_parameter("whi", [P, 6, H], f8, isOutput=False)
    wlo = nc.declare_dram_parameter("wlo", [P, 6, H], f8, isOutput=False)
    if not zero_bias:
        b0s = nc.declare_dram_parameter("b0s", [P, 2, 1], f32, isOutput=False)
        b1s = nc.declare_dram_parameter("b1s", [P, 2, 1], f32, isOutput=False)
        b2s = nc.declare_dram_parameter("b2s", [P, 2, 1], f32, isOutput=False)
        bouts = nc.declare_dram_parameter("bouts", [P, 6, 1], f32, isOutput=False)
    yT = nc.declare_dram_parameter("yT", [P, 6, N], bf, isOutput=True)

    with TileContext(nc) as tc:
        with (
            tc.tile_pool(name="weights", bufs=1) as wpool,
            tc.tile_pool(name="xin", bufs=2) as xpool,
            tc.tile_pool(name="xbin", bufs=2) as xbpool,
            tc.tile_pool(name="hbf", bufs=2) as hbpool,
            tc.tile_pool(name="hq", bufs=2) as hqpool,
            tc.tile_pool(name="bern", bufs=2) as bernpool,
            tc.tile_pool(name="bhi", bufs=2) as bhipool,
            tc.tile_pool(name="blo", bufs=2) as blopool,
            tc.tile_pool(name="yout", bufs=4) as ypool,
            tc.tile_pool(name="psum", bufs=4, space="PSUM") as psum,
        ):
            # ---- loads (HWDGE serializes ~630ns per issue: issue in
            # consumption order; first two ride separate queues to overlap
            # queue-side setup) -------------------------------------------
            # first weight/x pair-chunks ride first (as standalone tiles so
            # dependency tracking is per-DMA) so MM0 starts ~3.5us
            w0a = wpool.tile([P, 2, HD], f8)
            nc.sync.dma_start(out=w0a[:], in_=w0q[:, 0:2, :])

            def load_xq(t):
                xt = xpool.tile([P, 6, NTILE], f8, tag=f"xq{t % 2}", name=f"xq{t}")
                nc.sync.dma_start(out=xt[:], in_=xq[:, :, ts(t, NTILE)])
                return xt

            def load_xb(t):
                xt = xbpool.tile([P, 6, NTILE], bf, tag=f"xb{t % 2}", name=f"xb{t}")
                nc.sync.dma_start(out=xt[:], in_=xb[:, :, ts(t, NTILE)])
                return xt

            xq0a = xpool.tile([P, 2, NTILE], f8, tag="xq0a", name="xq0a")
            nc.scalar.dma_start(out=xq0a[:], in_=xq[:, 0:2, ts(0, NTILE)])
            w0b = wpool.tile([P, 4, HD], f8)
            nc.scalar.dma_start(out=w0b[:], in_=w0q[:, 2:6, :])
            xq0b = xpool.tile([P, 4, NTILE], f8, tag="xq0b", name="xq0b")
            nc.sync.dma_start(out=xq0b[:], in_=xq[:, 2:6, ts(0, NTILE)])
            if not zero_bias:
                bias_sb = []
                for nm, par in (("b0", b0s), ("b1", b1s), ("b2", b2s)):
                    t = wpool.tile([P, 2, 1], f32, name=nm)
                    nc.scalar.dma_start(out=t[:], in_=par[:, :, :])
                    bias_sb.append(t)
                bout_sb = wpool.tile([P, 6, 1], f32)
                nc.scalar.dma_start(out=bout_sb[:], in_=bouts[:, :, :])
            xq1 = load_xq(1)
            w1_sb = wpool.tile([P, 8, HD], f8)
            nc.sync.dma_start(out=w1_sb[:], in_=w1q[:, :, :])
            xq2 = load_xq(2)
            w2_sb = wpool.tile([P, 10, HD], f8)
            nc.sync.dma_start(out=w2_sb[:], in_=w2q[:, :, :])
            xq3 = load_xq(3)
            xb0 = load_xb(0)
            whi_sb = wpool.tile([P, 6, H], f8)
            nc.sync.dma_start(out=whi_sb[:], in_=whi[:, :, :])
            wlo_sb = wpool.tile([P, 6, H], f8)
            nc.sync.dma_start(out=wlo_sb[:], in_=wlo[:, :, :])
            xb1 = load_xb(1)
            xb2 = load_xb(2)
            xb3 = load_xb(3)

            # ---- per-tile stages ----------------------------------------
            def h_stages(t, xpairs):
                """Returns (h_bf, [stage0_thunk, stage1_thunk, stage2_thunk])
                so tiles' stages can be emitted interleaved."""
                h_bf = hbpool.tile([P, 6, NTILE], bf, tag="hbf", name=f"hbf{t}")
                h_hi = hqpool.tile([P, 6, NTILE], f8, tag="hhi", name=f"hhi{t}")

                def stage(s, wpairs, extra):
                    ps = psum.tile([P, 2, NTILE], f32, tag="ps", name="psh")
                    pairs = [(w, wk, x, xk)
                             for (w, wk), (x, xk) in zip(wpairs, xpairs)] + extra
                    np_ = len(pairs)
                    # pair-outer: each K-pair is consumed as its DMA lands
                    for pi, (wsb, wk, rhs, rk) in enumerate(pairs):
                        for m in range(2):
                            nc.tensor.matmul(
                                ps[:, m, :],
                                lhsT=wsb[:, wk:wk + 2, ts(m, P)],
                                rhs=rhs[:, rk:rk + 2, :],
                                start=(pi == 0), stop=(pi == np_ - 1),
                                perf_mode=DR,
                            )
                    if zero_bias:
                        nc.scalar.activation(
                            h_bf[:, 2 * s:2 * s + 2, :], ps[:, :, :],
                            AF.Relu, bias=0.0, scale=a0 * inv_sw[s],
                        )
                    else:
                        for m in range(2):
                            nc.scalar.activation(
                                h_bf[:, 2 * s + m, :], ps[:, m, :],
                                AF.Relu, bias=bias_sb[s][:, m, :],
                                scale=a0 * inv_sw[s],
                            )
                    if s < 2:
                        # h2's fp8 view is never consumed (only h0/h1 feed
                        # later matmuls; bern reads h_bf)
                        nc.vector.tensor_copy(
                            h_hi[:, 2 * s:2 * s + 2, :],
                            h_bf[:, 2 * s:2 * s + 2, :],
                        )

                thunks = [
                    lambda: stage(0, [(w0a, 0), (w0b, 0), (w0b, 2)], []),
                    lambda: stage(1, [(w1_sb, 0), (w1_sb, 2), (w1_sb, 4)],
                                  [(w1_sb, 6, h_hi, 0)]),
                    lambda: stage(2, [(w2_sb, 0), (w2_sb, 2), (w2_sb, 4)],
                                  [(w2_sb, 6, h_hi, 0), (w2_sb, 8, h_hi, 2)]),
                ]
                return h_bf, thunks

            def bern_make(t, xbt, h_bf, lo_dve=False):
                """bern at bf16, then hi/lo fp8 split (scale S).
                add/hi on DVE; lo subtract splits DVE/Pool halves (all-DVE
                for an early tile whose OUT groups start soon after)."""
                bern = bernpool.tile([P, 6, NTILE], bf, tag="bern", name=f"bern{t}")
                nc.vector.tensor_add(bern[:], xbt[:], h_bf[:])
                bhi = bhipool.tile([P, 6, NTILE], f8, tag="bhi", name=f"bhi{t}")
                nc.vector.tensor_copy(bhi[:], bern[:])
                blo = blopool.tile([P, 6, NTILE], f8, tag="blo", name=f"blo{t}")
                if lo_dve:
                    nc.vector.tensor_sub(blo[:], bern[:], bhi[:])
                else:
                    nc.vector.tensor_sub(
                        blo[:, 0:3, :], bern[:, 0:3, :], bhi[:, 0:3, :])
                    nc.gpsimd.tensor_sub(
                        blo[:, 3:6, :], bern[:, 3:6, :], bhi[:, 3:6, :])
                return bhi, blo

            def out_mm(t, g, bhl):
                """2 output banks; DR fp8 3-term: hi@Whi, hi@Wlo, lo@Whi.
                lo terms come last so the Pool/DVE subtract can trail."""
                bhi, blo = bhl
                ps = psum.tile([P, 2, NTILE], f32, tag="ps", name="pso")
                for mb in range(2):
                    m = 2 * g + mb
                    ti = 0
                    for wsb, rhs in ((whi_sb, bhi), (wlo_sb, bhi), (whi_sb, blo)):
                        for j in range(3):
                            nc.tensor.matmul(
                                ps[:, mb, :],
                                lhsT=wsb[:, 2 * j:2 * j + 2, ts(m, P)],
                                rhs=rhs[:, 2 * j:2 * j + 2, :],
                                start=(ti == 0), stop=(ti == 8),
                                perf_mode=DR,
                            )
                            ti += 1
                return ps

            def out_cast(t, g, ps, finale=False):

                out_scale = inv_swo / S

                def cast_store(sl, cols=None, eng=None):
                    width = sl.stop - sl.start
                    c0, c1 = (0, NTILE) if cols is None else cols
                    yt = ypool.tile([P, width, c1 - c0], bf,
                                    tag=f"yt{g}{sl.start}{c0}", name=f"yt{t}{g}")
                    if zero_bias and t == 3:
                        # last tile: ACT still trails earlier groups' casts;
                        # DVE is idle by now so the tail shrinks
                        nc.vector.tensor_scalar_mul(
                            yt[:], ps[:, sl, c0:c1], out_scale
                        )
                    elif zero_bias:
                        nc.scalar.activation(
                            yt[:], ps[:, sl, c0:c1], AF.Copy, bias=0.0,
                            scale=out_scale,
                        )
                    else:
                        for mb in range(width):
                            nc.scalar.activation(
                                yt[:, mb, :], ps[:, sl.start + mb, c0:c1],
                                AF.Identity,
                                bias=bout_sb[:, 2 * g + sl.start + mb, :],
                                scale=out_scale,
                            )
                    (eng or nc.sync).dma_start(
                        out=yT[:, 2 * g + sl.start:2 * g + sl.stop,
                               t * NTILE + c0:t * NTILE + c1],
                        in_=yt[:],
                    )

                if finale:
                    # last tile+group: per-bank cast+store so bank 0's chain
                    # overlaps bank 1's matmuls (3-way splits serialize on
                    # the single HWDGE slot and come out slower)
                    cast_store(slice(0, 1))
                    cast_store(slice(1, 2), eng=nc.scalar)
                else:
                    cast_store(slice(0, 2))

            # ---- pipeline -----------------------------------------------
            # all H stages first (ACT queue stays a pure h-chain; an OUT cast
            # queued before h(t) ops would stall them on OUT psums), with
            # tiles' stages interleaved so the PE never queues behind a
            # chain-blocked h-part of a single tile
            hb0, st0 = h_stages(0, [(xq0a, 0), (xq0b, 0), (xq0b, 2)])
            hb1, st1 = h_stages(1, [(xq1, 0), (xq1, 2), (xq1, 4)])
            hb2, st2 = h_stages(2, [(xq2, 0), (xq2, 2), (xq2, 4)])
            hb3, st3 = h_stages(3, [(xq3, 0), (xq3, 2), (xq3, 4)])
            st = {0: st0, 1: st1, 2: st2, 3: st3}
            berns = {}

            def emit(t, s):
                st[t][s]()
                if s == 2:
                    berns[t] = bern_make(
                        t, (xb0, xb1, xb2, xb3)[t], (hb0, hb1, hb2, hb3)[t],
                        lo_dve=False,
                    )

            for t, s in ((0, 0), (1, 0), (0, 1), (2, 0), (1, 1), (0, 2),
                         (3, 0), (2, 1), (1, 2), (3, 1)):
                emit(t, s)
            # tile 0's OUT matmuls interleave into the late H phase as PE
            # filler; their casts are emitted only after the last h op so the
            # ACT queue is never head-of-line-blocked by an OUT psum wait
            emit(2, 2)
            ps00 = out_mm(0, 0, berns[0])
            emit(3, 2)
            ps01 = out_mm(0, 1, berns[0])
            out_cast(0, 0, ps00)
            ps02 = out_mm(0, 2, berns[0])
            out_cast(0, 1, ps01)
            out_cast(0, 2, ps02)
            for t in range(1, NT):
                for g in range(3):
                    out_cast(t, g, out_mm(t, g, berns[t]),
                             finale=(t == 3 and g == 2))  # noqa: C901

    return nc


def _prep_inputs(inputs: dict, a0: float):
    """Quantize + rearrange all tensors on host (RNE via ml_dtypes)."""
    f = np.float32
    x = np.asarray(inputs["x"], f)

    def chunk_pm(a, nch):
        # [rows, cols] -> [128, nch, cols], rows = nch chunks of 128
        r, c = a.shape
        return np.ascontiguousarray(
            a.reshape(nch, P, c).transpose(1, 0, 2))

    Ws = [np.asarray(inputs[k], f) for k in ("W0", "W1", "W2")]
    sw = [WS / max(np.abs(W).max(), 1e-30) for W in Ws]
    w0q = chunk_pm((Ws[0] * sw[0]).astype(F8), 6)
    w1q = chunk_pm((Ws[1] * sw[1]).astype(F8), 8)
    w2q = chunk_pm((Ws[2] * sw[2]).astype(F8), 10)
    Wout = np.asarray(inputs["Wout"], f)
    swo = WS / max(np.abs(Wout).max(), 1e-30)
    whi_full = (Wout * swo).astype(F8)
    wlo_full = (Wout * swo - whi_full.astype(f)).astype(F8)

    shared = {
        "w0q": w0q, "w1q": w1q, "w2q": w2q,
        "whi": chunk_pm(whi_full, 6), "wlo": chunk_pm(wlo_full, 6),
    }
    bs = [np.asarray(inputs[k], f) for k in ("b0", "b1", "b2")]
    zero_bias = all(not np.any(b) for b in bs) and not np.any(
        np.asarray(inputs["bout"], f))
    if not zero_bias:
        for i, b in enumerate(bs):
            shared[f"b{i}s"] = np.ascontiguousarray(
                (S * a0 * b).reshape(2, P, 1).transpose(1, 0, 2)).astype(f)
        shared["bouts"] = np.ascontiguousarray(
            np.asarray(inputs["bout"], f).reshape(6, P, 1).transpose(1, 0, 2))

    in_maps = []
    for i in range(B):
        xT = np.ascontiguousarray(x[i].T) * S
        in_maps.append({
            "xq": chunk_pm(xT.astype(F8), 6),
            "xb": chunk_pm(xT.astype(BF16), 6),
            **shared,
        })
    inv_sw = tuple(float(1.0 / s) for s in sw)
    return in_maps, inv_sw, zero_bias, float(1.0 / swo)


def _run_fp8(inputs: dict, a0: float, trace: bool = False):
    from concourse.bass_utils import run_bass_kernel_spmd

    in_maps, inv_sw, zero_bias, inv_swo = _prep_inputs(inputs, a0)
    key = ("c4", round(a0, 12), tuple(round(v, 10) for v in inv_sw),
           round(inv_swo, 10), zero_bias)
    if key not in _CACHE:
        _CACHE[key] = _build_fp8_bass(a0, inv_sw, zero_bias, inv_swo)
    nc = _CACHE[key]

    res = run_bass_kernel_spmd(nc, in_maps, list(range(B)), trace=trace)
    f = np.float32
    out = np.empty((B, N, H), f)
    for i in range(B):
        yt = np.asarray(res.results[i]["yT"])  # [128, 6, 2048] bf16
        out[i] = yt.astype(f).transpose(1, 0, 2).reshape(H, N).T
    import jax

    jax.clear_caches()
    return np.ascontiguousarray(out, f), res


# ---------------------------------------------------------------------------
# General fallback (arbitrary coe): full reference computation in jax.
# ---------------------------------------------------------------------------


def _fallback_jax(inputs: dict) -> np.ndarray:
    import jax
    import jax.numpy as jnp

    def norm_adj(adj):
        A = (adj > 0).astype(adj.dtype)
        deg = A.sum(-1)
        dis = jnp.where(deg > 0, jax.lax.rsqrt(jnp.maximum(deg, 1e-12)), 0.0)
        return dis[..., :, None] * A * dis[..., None, :]

    def bern_conv(x, Ahat, coe, W, bvec):
        h = x @ W + bvec
        T = jax.nn.relu(coe)
        binom = jnp.asarray(
            [comb(K, k) / (2.0**K) for k in range(K + 1)], dtype=x.dtype
        )
        c = binom * T
        mm = lambda v: jnp.einsum("bij,bjh->bih", Ahat, v)
        tmp = [h]
        for _ in range(K):
            t = tmp[-1]
            tmp.append(t + mm(t))
        Lv = lambda v: v - mm(v)
        acc = c[K] * tmp[0]
        for i in range(K - 1, 0, -1):
            acc = Lv(acc) + c[i] * tmp[K - i]
        return c[0] * tmp[K] + Lv(acc)

    adj = jnp.asarray(inputs["adj"])
    x = jnp.asarray(inputs["x"])
    coe = jnp.asarray(inputs["coe"])
    Ahat = norm_adj(adj)
    h0 = jax.nn.relu(bern_conv(x, Ahat, coe, inputs["W0"], inputs["b0"]))
    h1 = jax.nn.relu(
        bern_conv(jnp.concatenate([x, h0], -1), Ahat, coe, inputs["W1"], inputs["b1"])
    )
    h2 = jax.nn.relu(
        bern_conv(
            jnp.concatenate([x, h0, h1], -1), Ahat, coe, inputs["W2"], inputs["b2"]
        )
    )
    bern = jnp.concatenate([h0, h1, h2], -1) + x
    out = bern @ jnp.asarray(inputs["Wout"]) + jnp.asarray(inputs["bout"])
    return np.asarray(out, np.float32)


def _collapsible(inputs: dict):
    if np.asarray(inputs["x"]).shape != (B, N, H):
        return None
    coe = np.asarray(inputs["coe"], np.float64)
    if coe.shape != (K + 1,):
        return None
    a = _monomial_coeffs(coe)
    if np.max(np.abs(a[1:])) <= 1e-12 * max(1.0, abs(a[0])):
        return float(a[0])
    return None


def kernel(**inputs) -> np.ndarray:
    a0 = _collapsible(inputs)
    if a0 is None or a0 <= 0.0:
        return _fallback_jax(inputs)
    out, _ = _run_fp8(inputs, a0)
    return out

